# revision 15
# baseline (speedup 1.0000x reference)
"""Multi-head attention (B=2, S=4096, D=512, H=8) on 8 Trainium2 NeuronCores.

Sharding: batch x head-pair parallelism. Core c handles batch b = c // 4 and
heads {2*(c%4), 2*(c%4)+1} (128 contiguous rows of the QKV projection
weights, Megatron column-parallel; Wo row-parallel with the partial-sum
reduction done on the host at gather time).

Per-core device program (identical on all cores, different data):
  1. Project K, Q: khT/qhT [128(hd), 4096] = W @ x.T via 4 contraction chunks.
     The 1/sqrt(DK) score scale and biases are folded into Wq/bq host-side.
  2. Project V the same way, then PE-transpose to vh [4096(s), 64] per head,
     augmented with a ones column (65th) so the AV matmul also produces the
     softmax denominator.
  3. Attention, per 512-wide query block, per head, per 128-wide key tile:
     scoresT [128(k), 512(q)] = kh_tile @ qh_block (PE, fp32r)
     expT = exp(scoresT)                            (ACT, PSUM->SBUF)
     av [65, 512] += vh_aug_tile.T @ expT           (PE accumulate; row 64 =
                                                     sum_k exp = denominator)
  4. Normalize: recip(denominator) (DVE), broadcast across 64 partitions via
     a ones-column PE matmul, multiply (DVE) -> avn [64, 512] per head.
  5. Output projection: out[s,:] += avn_h.T @ WoT_h for both heads (K=64
     accumulating matmuls), DVE-evacuate, DMA to HBM.

Host gathers: out[b] = sum of the 4 per-core partials + bv @ Wo.T + bo.
"""

import numpy as np

import concourse.bass as bass
import concourse.mybir as mybir
import concourse.tile as tile
from concourse import bacc
from concourse.bass_utils import run_bass_kernel_spmd
from concourse.masks import make_identity

F32 = mybir.dt.float32
F32R = mybir.dt.float32r
EXP = mybir.ActivationFunctionType.Exp
ADD = mybir.AluOpType.add

B, S, D, H = 2, 4096, 512, 8
DK = D // H          # 64
HPC = 2              # heads per core
HD = HPC * DK        # 128 head-dims per core
N_CORES = 8
QB = 512             # query block (matmul free dim)
KT = 128             # key tile (partition dim)
NQB = S // QB        # 8
NKT = S // KT        # 32
NCH = D // 128       # 4 contraction chunks for the projections


def r32(ap):
    return ap.bitcast(F32R)


def mha_tile_kernel(tc, out_ap, ins, seq=S, dbg=None):
    """Emit the per-core MHA program. `ins` maps name -> DRAM AP."""
    nc = tc.nc
    nqb, nkt = seq // QB, seq // KT

    xq, xk, xv = ins["qt"], ins["kt"], ins["vt"]
    const = tc.alloc_tile_pool(name="const", bufs=1)
    sb = tc.alloc_tile_pool(name="sb", bufs=2)
    pps = tc.alloc_tile_pool(name="pps", bufs=2, space="PSUM")

    # --- constants ---
    wq_sb = const.tile([128, NCH, 128], F32R, tag="wq", name="wq_sb")
    wk_sb = const.tile([128, NCH, 128], F32R, tag="wk", name="wk_sb")
    wv_sb = const.tile([128, NCH, 128], F32R, tag="wv", name="wv_sb")
    for w_sb, name in ((wq_sb, "wq"), (wk_sb, "wk"), (wv_sb, "wv")):
        nc.sync.dma_start(w_sb, ins[name].rearrange("(c p) m -> p c m", p=128))
    wo0_sb = const.tile([64, QB], F32R, tag="wo0", name="wo0_sb")
    wo1_sb = const.tile([64, QB], F32R, tag="wo1", name="wo1_sb")
    nc.sync.dma_start(wo0_sb, ins["wo0"])
    nc.sync.dma_start(wo1_sb, ins["wo1"])
    bq_sb = const.tile([128, 1], F32, tag="bq", name="bq_sb")
    bk_sb = const.tile([128, 1], F32, tag="bk", name="bk_sb")
    nc.sync.dma_start(bq_sb, ins["bq"])
    nc.sync.dma_start(bk_sb, ins["bk"])

    ident = const.tile([128, 128], F32, tag="ident", name="ident")
    make_identity(nc, ident)
    ones_sb = const.tile([128, 64], F32, tag="ones", name="ones_sb")
    nc.vector.memset(ones_sb, 1.0)

    # --- persistent activations ---
    qhT = const.tile([128, seq], F32R, tag="qhT", name="qhT")
    khT = const.tile([128, seq], F32R, tag="khT", name="khT")
    # vh per head: nkt tiles of [128, 65]; column 64 is the ones column.
    vh = [
        const.tile([128, nkt * 65], F32R, tag=f"vh{h}", name=f"vh{h}")
        for h in range(HPC)
    ]
    for h in range(HPC):
        ones_col = vh[h].rearrange("p (j c) -> p j c", c=65)[:, :, 64]
        nc.vector.tensor_copy(out=ones_col, in_=ones_sb[:, 0:nkt])

    # --- projections of K and Q: dstT[hd, s] = W @ x.T (+ bias) ---
    def project_T(x_dram, w_sb, bias, dstT):
        xc = [
            sb.tile([128, seq], F32R, tag="xchunk", bufs=NCH + 1, name=f"xc{c}")
            for c in range(NCH)
        ]
        for c in range(NCH):
            nc.sync.dma_start(xc[c], x_dram[c * 128 : (c + 1) * 128, :])
        for qb in range(nqb):
            acc = pps.tile([128, QB], F32, tag="proj", bufs=4, name="prj")
            for c in range(NCH):
                nc.tensor.matmul(
                    acc,
                    lhsT=w_sb[:, c, :],
                    rhs=xc[c][:, qb * QB : (qb + 1) * QB],
                    start=(c == 0),
                    stop=(c == NCH - 1),
                )
            dst = dstT[:, qb * QB : (qb + 1) * QB]
            if bias is None:
                nc.vector.tensor_copy(out=dst, in_=acc)
            else:
                nc.vector.tensor_scalar(dst, acc, bias[:, 0:1], None, ADD)

    project_T(xk, wk_sb, bk_sb, khT)
    project_T(xq, wq_sb, bq_sb, qhT)

    # --- V: project to vhT then PE-transpose into vh[s, d] tiles ---
    vhT = sb.tile([128, seq], F32, tag="vhT", bufs=1, name="vhT")
    project_T(xv, wv_sb, None, vhT)
    for j in range(nkt):
        tp = pps.tile([128, 128], F32, tag="tp", bufs=2, name="tp")
        nc.tensor.transpose(tp, vhT[:, j * 128 : (j + 1) * 128], ident)
        for h in range(HPC):
            nc.vector.tensor_copy(
                out=vh[h][:, j * 65 : j * 65 + 64],
                in_=tp[:, h * 64 : (h + 1) * 64],
            )
    pps.release()
    ps = tc.alloc_tile_pool(name="ps", bufs=2, space="PSUM")

    if dbg is not None:
        nc.sync.dma_start(dbg["qhT"], qhT.bitcast(F32))
        nc.sync.dma_start(dbg["khT"], khT.bitcast(F32))
        nc.sync.dma_start(dbg["vh0"], vh[0].bitcast(F32))
        nc.sync.dma_start(dbg["vh1"], vh[1].bitcast(F32))

    # --- attention + output projection, per query block ---
    for qb in range(nqb):
        qsl = slice(qb * QB, (qb + 1) * QB)
        av = [
            ps.tile([128, QB], F32, tag=f"av{h}", bufs=1, name=f"av{h}")
            for h in range(HPC)
        ]
        for kt_i in range(nkt):
            for h in range(HPC):
                hp = slice(h * 64, (h + 1) * 64)
                sc = ps.tile([128, QB], F32, tag="sc", bufs=3, name="sc")
                nc.tensor.matmul(
                    sc,
                    lhsT=khT[hp, kt_i * KT : (kt_i + 1) * KT],
                    rhs=qhT[hp, qsl],
                    start=True,
                    stop=True,
                )
                ex = sb.tile([128, QB], F32R, tag="ex", bufs=6, name="ex")
                nc.scalar.activation(ex, sc, EXP)
                nc.tensor.matmul(
                    av[h][0:65, :],
                    lhsT=vh[h][:, kt_i * 65 : kt_i * 65 + 65],
                    rhs=ex,
                    start=(kt_i == 0),
                    stop=(kt_i == nkt - 1),
                )
        # normalize each head: avn = av[0:64] * (1/av[64]) broadcast
        avn = []
        for h in range(HPC):
            r_sb = sb.tile([128, QB], F32, tag="r", bufs=2, name="r_sb")
            nc.vector.reciprocal(out=r_sb[64:65, :], in_=av[h][64:65, :])
            bc = ps.tile([64, QB], F32, tag="bc", bufs=1, name="bc")
            nc.tensor.matmul(
                bc,
                lhsT=ones_sb[64:65, :],
                rhs=r_sb[64:65, :],
                start=True,
                stop=True,
            )
            bc_sb = sb.tile([64, QB], F32, tag="bcs", bufs=2, name="bc_sb")
            nc.vector.tensor_copy(out=bc_sb, in_=bc)
            if dbg is not None:
                den_sb = sb.tile([128, QB], F32, tag="den", bufs=2, name="den_sb")
                nc.vector.tensor_copy(out=den_sb[64:65, :], in_=av[h][64:65, :])
                nc.sync.dma_start(
                    dbg[f"den{h}"][qb : qb + 1, :], den_sb[64:65, :]
                )
                nc.sync.dma_start(dbg[f"r{h}"][qb : qb + 1, :], r_sb[64:65, :])
                nc.sync.dma_start(
                    dbg[f"bc{h}"][qb * 64 : (qb + 1) * 64, :], bc_sb
                )
                ex_dump = sb.tile([128, QB], F32, tag="exd", bufs=2, name="ex_dump")
                nc.vector.tensor_copy(out=ex_dump[0:65, :], in_=av[h][0:65, :])
                nc.sync.dma_start(
                    dbg[f"av{h}"][qb * 65 : (qb + 1) * 65, :], ex_dump[0:65, :]
                )
            a = sb.tile([64, QB], F32R, tag=f"avn{h}", bufs=2, name=f"avn{h}")
            nc.vector.tensor_tensor(a, av[h][0:64, :], bc_sb, mybir.AluOpType.mult)
            avn.append(a)
        # output projection: out rows = avn.T @ WoT, accumulated over heads
        for st in range(QB // 128):
            ssl = slice(st * 128, (st + 1) * 128)
            op = ps.tile([128, QB], F32, tag="op", bufs=2, name="op")
            nc.tensor.matmul(
                op, lhsT=avn[0][:, ssl], rhs=wo0_sb, start=True, stop=False
            )
            nc.tensor.matmul(
                op, lhsT=avn[1][:, ssl], rhs=wo1_sb, start=False, stop=True
            )
            ost = sb.tile([128, QB], F32, tag="ost", bufs=3, name="ost")
            nc.vector.tensor_copy(out=ost, in_=op)
            nc.sync.dma_start(out_ap[qb * QB + st * 128 : qb * QB + (st + 1) * 128, :], ost)

    ps.release()
    sb.release()
    const.release()


def build_bass(seq=S, debug_outs=False):
    nc = bacc.Bacc(
        "TRN2",
        debug=False,
        enable_asserts=False,
        target_bir_lowering=False,
    )
    ins = {}
    shapes = {
        "qt": (D, seq), "kt": (D, seq), "vt": (D, seq),
        "wq": (D, HD), "wk": (D, HD), "wv": (D, HD),
        "wo0": (64, D), "wo1": (64, D),
        "bq": (HD, 1), "bk": (HD, 1),
    }
    f32r_names = {"qt", "kt", "vt", "wq", "wk", "wv", "wo0", "wo1"}
    for name, shape in shapes.items():
        dt = F32R if name in f32r_names else F32
        ins[name] = nc.dram_tensor(name, list(shape), dt, kind="ExternalInput").ap()
    out = nc.dram_tensor("out", [seq, D], F32, kind="ExternalOutput").ap()
    dbg = None
    if debug_outs:
        nkt, nqb = seq // KT, seq // QB
        dbg_shapes = {
            "qhT": (128, seq), "khT": (128, seq),
            "vh0": (128, nkt * 65), "vh1": (128, nkt * 65),
            "den0": (nqb, QB), "den1": (nqb, QB),
            "r0": (nqb, QB), "r1": (nqb, QB),
            "bc0": (nqb * 64, QB), "bc1": (nqb * 64, QB),
            "av0": (nqb * 65, QB), "av1": (nqb * 65, QB),
        }
        dbg = {
            n: nc.dram_tensor(f"dbg_{n}", list(sh), F32, kind="ExternalOutput").ap()
            for n, sh in dbg_shapes.items()
        }
    with tile.TileContext(nc) as tc:
        mha_tile_kernel(tc, out, ins, seq=seq, dbg=dbg)
    nc.compile()
    return nc


def shard_inputs(q, k, v, Wq, bq, Wk, bk, Wv, bv, Wo, bo, seq=S):
    """Host-side shard prep. Returns (in_maps, const_vec)."""
    scale = 1.0 / np.sqrt(np.float32(DK))
    in_maps = []
    for c in range(N_CORES):
        b = c // 4
        rows = slice(128 * (c % 4), 128 * (c % 4) + 128)
        in_maps.append({
            "qt": np.ascontiguousarray(q[b].T),
            "kt": np.ascontiguousarray(k[b].T),
            "vt": np.ascontiguousarray(v[b].T),
            "wq": np.ascontiguousarray((Wq[rows, :] * scale).T),
            "wk": np.ascontiguousarray(Wk[rows, :].T),
            "wv": np.ascontiguousarray(Wv[rows, :].T),
            "wo0": np.ascontiguousarray(Wo[:, rows][:, 0:64].T),
            "wo1": np.ascontiguousarray(Wo[:, rows][:, 64:128].T),
            "bq": np.ascontiguousarray((bq[rows] * scale).reshape(HD, 1)),
            "bk": np.ascontiguousarray(bk[rows].reshape(HD, 1)),
        })
    const_vec = (bv @ Wo.T + bo).astype(np.float32)
    return in_maps, const_vec


_NC_CACHE = {}


def run(inputs, seq=S, trace=False, trace_kwargs=None):
    if seq not in _NC_CACHE:
        _NC_CACHE[seq] = build_bass(seq=seq)
    nc = _NC_CACHE[seq]
    in_maps, const_vec = shard_inputs(**inputs, seq=seq)
    res = run_bass_kernel_spmd(
        nc,
        in_maps,
        core_ids=list(range(N_CORES)),
        trace=trace,
        **(trace_kwargs or {}),
    )
    out = np.zeros((B, seq, D), dtype=np.float32)
    for c in range(N_CORES):
        out[c // 4] += res.results[c]["out"]
    out += const_vec[None, None, :]
    return out, res


def kernel(**inputs):
    out, _ = run(inputs)
    return out


# revision 19
# speedup vs baseline: 1.4967x; 1.4967x over previous
"""Multi-head attention (B=2, S=4096, D=512, H=8) on 8 Trainium2 NeuronCores.

Sharding: batch x head-pair parallelism. Core c handles batch b = c // 4 and
heads {2*(c%4), 2*(c%4)+1} (128 contiguous rows of the QKV projection
weights, Megatron column-parallel; Wo row-parallel with the partial-sum
reduction done on the host at gather time).

Per-core device program (identical on all cores, different data; matmul
operands in bf16, all accumulation in fp32 PSUM):
  1. Project K, Q: khT/qhT [128(hd), 4096] = W @ x.T via 4 contraction chunks.
     The 1/sqrt(DK) score scale and biases are folded into Wq/bq host-side.
  2. Project V the same way, then PE-transpose to vh [4096(s), 64] per head,
     augmented with a ones column (65th) so the AV matmul also produces the
     softmax denominator.
  3. Attention, per 512-wide query block, per head, per pair of 128-wide key
     tiles (paired so each ACT exp call covers 1024 elements of free dim):
     scoresT [128(k), 512(q)] = kh_tile @ qh_block        (PE)
     expT = exp(scoresT) over both tiles of the pair      (ACT, PSUM->SBUF)
     av [65, 512] += vh_aug_tile.T @ expT                 (PE accumulate;
                                         row 64 = sum_k exp = denominator)
  4. Normalize: recip(denominator) (DVE), broadcast across 64 partitions
     (GpSimd), multiply (DVE) -> avn [64, 512] bf16 per head.
  5. Output projection: out[s,:] += avn_h.T @ WoT_h for both heads (K=64
     accumulating matmuls), DVE-evacuate, DMA to HBM.

Host gathers: out[b] = sum of the 4 per-core partials + bv @ Wo.T + bo.
"""

import ml_dtypes
import numpy as np

import concourse.mybir as mybir
import concourse.tile as tile
from concourse import bacc
from concourse.bass_utils import run_bass_kernel_spmd
from concourse.masks import make_identity

F32 = mybir.dt.float32
BF16 = mybir.dt.bfloat16
EXP = mybir.ActivationFunctionType.Exp
ADD = mybir.AluOpType.add
MULT = mybir.AluOpType.mult
NPBF16 = ml_dtypes.bfloat16

B, S, D, H = 2, 4096, 512, 8
DK = D // H          # 64
HPC = 2              # heads per core
HD = HPC * DK        # 128 head-dims per core
N_CORES = 8
QB = 512             # query block (matmul free dim)
KT = 128             # key tile (partition dim)
NCH = D // 128       # 4 contraction chunks for the projections


def mha_tile_kernel(tc, out_ap, ins, seq=S, dbg=None):
    """Emit the per-core MHA program. `ins` maps name -> DRAM AP."""
    nc = tc.nc
    nqb, nkt = seq // QB, seq // KT

    xq, xk, xv = ins["qt"], ins["kt"], ins["vt"]
    const = tc.alloc_tile_pool(name="const", bufs=1)
    sb = tc.alloc_tile_pool(name="sb", bufs=2)
    pps = tc.alloc_tile_pool(name="pps", bufs=2, space="PSUM")

    # --- constants ---
    wq_sb = const.tile([128, NCH, 128], BF16, tag="wq", name="wq_sb")
    wk_sb = const.tile([128, NCH, 128], BF16, tag="wk", name="wk_sb")
    wv_sb = const.tile([128, NCH, 128], BF16, tag="wv", name="wv_sb")
    for w_sb, name in ((wq_sb, "wq"), (wk_sb, "wk"), (wv_sb, "wv")):
        nc.sync.dma_start(w_sb, ins[name].rearrange("(c p) m -> p c m", p=128))
    wo0_sb = const.tile([64, QB], BF16, tag="wo0", name="wo0_sb")
    wo1_sb = const.tile([64, QB], BF16, tag="wo1", name="wo1_sb")
    nc.sync.dma_start(wo0_sb, ins["wo0"])
    nc.sync.dma_start(wo1_sb, ins["wo1"])
    bq_sb = const.tile([128, 1], F32, tag="bq", name="bq_sb")
    bk_sb = const.tile([128, 1], F32, tag="bk", name="bk_sb")
    nc.sync.dma_start(bq_sb, ins["bq"])
    nc.sync.dma_start(bk_sb, ins["bk"])

    ident = const.tile([128, 128], BF16, tag="ident", name="ident")
    make_identity(nc, ident)
    ones_sb = const.tile([128, 64], F32, tag="ones", name="ones_sb")
    nc.vector.memset(ones_sb, 1.0)

    # --- persistent activations ---
    qhT = const.tile([128, seq], BF16, tag="qhT", name="qhT")
    khT = const.tile([128, seq], BF16, tag="khT", name="khT")
    # vh per head: nkt tiles of [128, 65]; column 64 is the ones column.
    vh = [
        const.tile([128, nkt * 65], BF16, tag=f"vh{h}", name=f"vh{h}")
        for h in range(HPC)
    ]
    for h in range(HPC):
        ones_col = vh[h].rearrange("p (j c) -> p j c", c=65)[:, :, 64]
        nc.vector.tensor_copy(out=ones_col, in_=ones_sb[:, 0:nkt])

    # --- projections of K and Q: dstT[hd, s] = W @ x.T (+ bias) ---
    def project_T(x_dram, w_sb, bias, dstT):
        xc = [
            sb.tile([128, seq], BF16, tag="xchunk", bufs=NCH + 1, name=f"xc{c}")
            for c in range(NCH)
        ]
        for c in range(NCH):
            nc.sync.dma_start(xc[c], x_dram[c * 128 : (c + 1) * 128, :])
        for qb in range(nqb):
            acc = pps.tile([128, QB], F32, tag="proj", bufs=4, name="prj")
            for c in range(NCH):
                nc.tensor.matmul(
                    acc,
                    lhsT=w_sb[:, c, :],
                    rhs=xc[c][:, qb * QB : (qb + 1) * QB],
                    start=(c == 0),
                    stop=(c == NCH - 1),
                )
            dst = dstT[:, qb * QB : (qb + 1) * QB]
            if bias is None:
                nc.vector.tensor_copy(out=dst, in_=acc)
            else:
                nc.vector.tensor_scalar(dst, acc, bias[:, 0:1], None, ADD)

    project_T(xk, wk_sb, bk_sb, khT)
    project_T(xq, wq_sb, bq_sb, qhT)

    # --- V: project to vhT then PE-transpose into vh[s, d] tiles ---
    vhT = sb.tile([128, seq], BF16, tag="vhT", bufs=1, name="vhT")
    project_T(xv, wv_sb, None, vhT)
    for j in range(nkt):
        tp = pps.tile([128, 128], BF16, tag="tp", bufs=2, name="tp")
        nc.tensor.transpose(tp, vhT[:, j * 128 : (j + 1) * 128], ident)
        for h in range(HPC):
            nc.vector.tensor_copy(
                out=vh[h][:, j * 65 : j * 65 + 64],
                in_=tp[:, h * 64 : (h + 1) * 64],
            )
    pps.release()
    ps = tc.alloc_tile_pool(name="ps", bufs=2, space="PSUM")

    if dbg is not None:
        nc.sync.dma_start(dbg["qhT"], qhT)
        nc.sync.dma_start(dbg["khT"], khT)
        nc.sync.dma_start(dbg["vh0"], vh[0])
        nc.sync.dma_start(dbg["vh1"], vh[1])

    # --- attention + output projection, per query block ---
    for qb in range(nqb):
        qsl = slice(qb * QB, (qb + 1) * QB)
        av = [
            ps.tile([128, QB], F32, tag=f"av{h}", bufs=1, name=f"av{h}")
            for h in range(HPC)
        ]
        for ktp in range(nkt // 2):
            for h in range(HPC):
                hp = slice(h * 64, (h + 1) * 64)
                # two key tiles' scoresT in one 2-bank PSUM tile
                sc = ps.tile([128, 2 * QB], F32, tag="sc", bufs=2, name="sc")
                for half in range(2):
                    kt_i = 2 * ktp + half
                    nc.tensor.matmul(
                        sc[:, half * QB : (half + 1) * QB],
                        lhsT=khT[hp, kt_i * KT : (kt_i + 1) * KT],
                        rhs=qhT[hp, qsl],
                        start=True,
                        stop=True,
                    )
                ex = sb.tile([128, 2 * QB], BF16, tag="ex", bufs=4, name="ex")
                nc.scalar.activation(ex, sc, EXP)
                for half in range(2):
                    kt_i = 2 * ktp + half
                    nc.tensor.matmul(
                        av[h][0:65, :],
                        lhsT=vh[h][:, kt_i * 65 : kt_i * 65 + 65],
                        rhs=ex[:, half * QB : (half + 1) * QB],
                        start=(kt_i == 0),
                        stop=(kt_i == nkt - 1),
                    )
        # normalize each head: avn = av[0:64] * (1/av[64]) broadcast
        avn = []
        for h in range(HPC):
            r_sb = sb.tile([128, QB], F32, tag="r", bufs=2, name="r_sb")
            nc.vector.reciprocal(out=r_sb[64:65, :], in_=av[h][64:65, :])
            bc = ps.tile([64, QB], F32, tag="bc", bufs=1, name="bc")
            nc.tensor.matmul(
                bc,
                lhsT=ones_sb[64:65, :],
                rhs=r_sb[64:65, :],
                start=True,
                stop=True,
            )
            bc_sb = sb.tile([64, QB], F32, tag="bcs", bufs=2, name="bc_sb")
            nc.vector.tensor_copy(out=bc_sb, in_=bc)
            a = sb.tile([64, QB], BF16, tag=f"avn{h}", bufs=2, name=f"avn{h}")
            nc.vector.tensor_tensor(a, av[h][0:64, :], bc_sb, MULT)
            avn.append(a)
            if dbg is not None:
                den_sb = sb.tile([128, QB], F32, tag="den", bufs=2, name="den_sb")
                nc.vector.tensor_copy(out=den_sb[64:65, :], in_=av[h][64:65, :])
                nc.sync.dma_start(dbg[f"den{h}"][qb : qb + 1, :], den_sb[64:65, :])
                nc.sync.dma_start(dbg[f"r{h}"][qb : qb + 1, :], r_sb[64:65, :])
                nc.sync.dma_start(dbg[f"bc{h}"][qb * 64 : (qb + 1) * 64, :], bc_sb)
                ex_dump = sb.tile([128, QB], F32, tag="exd", bufs=2, name="ex_dump")
                nc.vector.tensor_copy(out=ex_dump[0:65, :], in_=av[h][0:65, :])
                nc.sync.dma_start(
                    dbg[f"av{h}"][qb * 65 : (qb + 1) * 65, :], ex_dump[0:65, :]
                )
        # output projection: out rows = avn.T @ WoT, accumulated over heads
        for st in range(QB // 128):
            ssl = slice(st * 128, (st + 1) * 128)
            op = ps.tile([128, QB], F32, tag="op", bufs=1, name="op")
            nc.tensor.matmul(
                op, lhsT=avn[0][:, ssl], rhs=wo0_sb, start=True, stop=False
            )
            nc.tensor.matmul(
                op, lhsT=avn[1][:, ssl], rhs=wo1_sb, start=False, stop=True
            )
            ost = sb.tile([128, QB], F32, tag="ost", bufs=3, name="ost")
            nc.vector.tensor_copy(out=ost, in_=op)
            nc.sync.dma_start(
                out_ap[qb * QB + st * 128 : qb * QB + (st + 1) * 128, :], ost
            )

    ps.release()
    sb.release()
    const.release()


def build_bass(seq=S, debug_outs=False):
    nc = bacc.Bacc(
        "TRN2",
        debug=False,
        enable_asserts=False,
        target_bir_lowering=False,
    )
    ins = {}
    shapes = {
        "qt": (D, seq), "kt": (D, seq), "vt": (D, seq),
        "wq": (D, HD), "wk": (D, HD), "wv": (D, HD),
        "wo0": (64, D), "wo1": (64, D),
        "bq": (HD, 1), "bk": (HD, 1),
    }
    bf16_names = {"qt", "kt", "vt", "wq", "wk", "wv", "wo0", "wo1"}
    for name, shape in shapes.items():
        dt = BF16 if name in bf16_names else F32
        ins[name] = nc.dram_tensor(name, list(shape), dt, kind="ExternalInput").ap()
    out = nc.dram_tensor("out", [seq, D], F32, kind="ExternalOutput").ap()
    dbg = None
    if debug_outs:
        nkt, nqb = seq // KT, seq // QB
        dbg_shapes = {
            "qhT": ((128, seq), BF16), "khT": ((128, seq), BF16),
            "vh0": ((128, nkt * 65), BF16), "vh1": ((128, nkt * 65), BF16),
            "den0": ((nqb, QB), F32), "den1": ((nqb, QB), F32),
            "r0": ((nqb, QB), F32), "r1": ((nqb, QB), F32),
            "bc0": ((nqb * 64, QB), F32), "bc1": ((nqb * 64, QB), F32),
            "av0": ((nqb * 65, QB), F32), "av1": ((nqb * 65, QB), F32),
        }
        dbg = {
            n: nc.dram_tensor(f"dbg_{n}", list(sh), dt, kind="ExternalOutput").ap()
            for n, (sh, dt) in dbg_shapes.items()
        }
    with tile.TileContext(nc) as tc:
        mha_tile_kernel(tc, out, ins, seq=seq, dbg=dbg)
    nc.compile()
    return nc


def shard_inputs(q, k, v, Wq, bq, Wk, bk, Wv, bv, Wo, bo, seq=S):
    """Host-side shard prep. Returns (in_maps, const_vec)."""
    scale = 1.0 / np.sqrt(np.float32(DK))
    q, k, v = (np.asarray(x, np.float32) for x in (q, k, v))
    Wq, bq, Wk, bk, Wv, bv, Wo, bo = (
        np.asarray(x, np.float32) for x in (Wq, bq, Wk, bk, Wv, bv, Wo, bo)
    )
    bf = lambda x: np.ascontiguousarray(x).astype(NPBF16)
    in_maps = []
    for c in range(N_CORES):
        b = c // 4
        rows = slice(128 * (c % 4), 128 * (c % 4) + 128)
        in_maps.append({
            "qt": bf(q[b].T),
            "kt": bf(k[b].T),
            "vt": bf(v[b].T),
            "wq": bf((Wq[rows, :] * scale).T),
            "wk": bf(Wk[rows, :].T),
            "wv": bf(Wv[rows, :].T),
            "wo0": bf(Wo[:, rows][:, 0:64].T),
            "wo1": bf(Wo[:, rows][:, 64:128].T),
            "bq": np.ascontiguousarray((bq[rows] * scale).reshape(HD, 1)),
            "bk": np.ascontiguousarray(bk[rows].reshape(HD, 1)),
        })
    const_vec = (bv @ Wo.T + bo).astype(np.float32)
    return in_maps, const_vec


_NC_CACHE = {}


def run(inputs, seq=S, trace=False, trace_kwargs=None):
    if seq not in _NC_CACHE:
        _NC_CACHE[seq] = build_bass(seq=seq)
    nc = _NC_CACHE[seq]
    in_maps, const_vec = shard_inputs(**inputs, seq=seq)
    res = run_bass_kernel_spmd(
        nc,
        in_maps,
        core_ids=list(range(N_CORES)),
        trace=trace,
        **(trace_kwargs or {}),
    )
    out = np.zeros((B, seq, D), dtype=np.float32)
    for c in range(N_CORES):
        out[c // 4] += res.results[c]["out"]
    out += const_vec[None, None, :]
    return out, res


def kernel(**inputs):
    out, _ = run(inputs)
    return out


# revision 20
# speedup vs baseline: 1.5138x; 1.0114x over previous
"""Multi-head attention (B=2, S=4096, D=512, H=8) on 8 Trainium2 NeuronCores.

Sharding: batch x head-pair parallelism. Core c handles batch b = c // 4 and
heads {2*(c%4), 2*(c%4)+1} (128 contiguous rows of the QKV projection
weights, Megatron column-parallel; Wo row-parallel with the partial-sum
reduction done on the host at gather time).

Per-core device program (identical on all cores, different data; matmul
operands in bf16, all accumulation in fp32 PSUM):
  1. Project K, Q: khT/qhT [128(hd), 4096] = W @ x.T via 4 contraction chunks.
     The 1/sqrt(DK) score scale and biases are folded into Wq/bq host-side.
  2. Project V the same way, then PE-transpose to vh [4096(s), 64] per head,
     augmented with a ones column (65th) so the AV matmul also produces the
     softmax denominator.
  3. Attention, per 512-wide query block, per head, per pair of 128-wide key
     tiles (paired so each ACT exp call covers 1024 elements of free dim):
     scoresT [128(k), 512(q)] = kh_tile @ qh_block        (PE)
     expT = exp(scoresT) over both tiles of the pair      (ACT, PSUM->SBUF)
     av [65, 512] += vh_aug_tile.T @ expT                 (PE accumulate;
                                         row 64 = sum_k exp = denominator)
  4. Normalize: recip(denominator) (DVE), broadcast across 64 partitions
     (GpSimd), multiply (DVE) -> avn [64, 512] bf16 per head.
  5. Output projection: out[s,:] += avn_h.T @ WoT_h for both heads (K=64
     accumulating matmuls), DVE-evacuate, DMA to HBM.

Host gathers: out[b] = sum of the 4 per-core partials + bv @ Wo.T + bo.
"""

import ml_dtypes
import numpy as np

import concourse.mybir as mybir
import concourse.tile as tile
from concourse import bacc
from concourse.bass_utils import run_bass_kernel_spmd
from concourse.masks import make_identity

F32 = mybir.dt.float32
BF16 = mybir.dt.bfloat16
EXP = mybir.ActivationFunctionType.Exp
ADD = mybir.AluOpType.add
MULT = mybir.AluOpType.mult
NPBF16 = ml_dtypes.bfloat16

B, S, D, H = 2, 4096, 512, 8
DK = D // H          # 64
HPC = 2              # heads per core
HD = HPC * DK        # 128 head-dims per core
N_CORES = 8
QB = 512             # query block (matmul free dim)
KT = 128             # key tile (partition dim)
NCH = D // 128       # 4 contraction chunks for the projections


def mha_tile_kernel(tc, out_ap, ins, seq=S, dbg=None):
    """Emit the per-core MHA program. `ins` maps name -> DRAM AP."""
    nc = tc.nc
    nqb, nkt = seq // QB, seq // KT

    xq, xk, xv = ins["qt"], ins["kt"], ins["vt"]
    const = tc.alloc_tile_pool(name="const", bufs=1)
    sb = tc.alloc_tile_pool(name="sb", bufs=2)
    pps = tc.alloc_tile_pool(name="pps", bufs=2, space="PSUM")

    # --- constants ---
    wq_sb = const.tile([128, NCH, 128], BF16, tag="wq", name="wq_sb")
    wk_sb = const.tile([128, NCH, 128], BF16, tag="wk", name="wk_sb")
    wv_sb = const.tile([128, NCH, 128], BF16, tag="wv", name="wv_sb")
    for w_sb, name in ((wq_sb, "wq"), (wk_sb, "wk"), (wv_sb, "wv")):
        nc.sync.dma_start(w_sb, ins[name].rearrange("(c p) m -> p c m", p=128))
    wo0_sb = const.tile([64, QB], BF16, tag="wo0", name="wo0_sb")
    wo1_sb = const.tile([64, QB], BF16, tag="wo1", name="wo1_sb")
    nc.sync.dma_start(wo0_sb, ins["wo0"])
    nc.sync.dma_start(wo1_sb, ins["wo1"])
    bq_sb = const.tile([128, 1], F32, tag="bq", name="bq_sb")
    bk_sb = const.tile([128, 1], F32, tag="bk", name="bk_sb")
    nc.sync.dma_start(bq_sb, ins["bq"])
    nc.sync.dma_start(bk_sb, ins["bk"])

    ident = const.tile([128, 128], BF16, tag="ident", name="ident")
    make_identity(nc, ident)
    ones_sb = const.tile([128, 64], F32, tag="ones", name="ones_sb")
    nc.vector.memset(ones_sb, 1.0)

    # --- persistent activations ---
    qhT = const.tile([128, seq], BF16, tag="qhT", name="qhT")
    khT = const.tile([128, seq], BF16, tag="khT", name="khT")
    # vh per head: nkt tiles of [128, 65]; column 64 is the ones column.
    vh = [
        const.tile([128, nkt * 65], BF16, tag=f"vh{h}", name=f"vh{h}")
        for h in range(HPC)
    ]
    for h in range(HPC):
        ones_col = vh[h].rearrange("p (j c) -> p j c", c=65)[:, :, 64]
        nc.vector.tensor_copy(out=ones_col, in_=ones_sb[:, 0:nkt])

    # --- projections of K and Q: dstT[hd, s] = W @ x.T (+ bias) ---
    def project_T(x_dram, w_sb, bias, dstT):
        xc = [
            sb.tile([128, seq], BF16, tag="xchunk", bufs=NCH + 1, name=f"xc{c}")
            for c in range(NCH)
        ]
        for c in range(NCH):
            nc.sync.dma_start(xc[c], x_dram[c * 128 : (c + 1) * 128, :])
        for qb in range(nqb):
            acc = pps.tile([128, QB], F32, tag="proj", bufs=4, name="prj")
            for c in range(NCH):
                nc.tensor.matmul(
                    acc,
                    lhsT=w_sb[:, c, :],
                    rhs=xc[c][:, qb * QB : (qb + 1) * QB],
                    start=(c == 0),
                    stop=(c == NCH - 1),
                )
            dst = dstT[:, qb * QB : (qb + 1) * QB]
            if bias is None:
                nc.vector.tensor_copy(out=dst, in_=acc)
            else:
                nc.vector.tensor_scalar(dst, acc, bias[:, 0:1], None, ADD)

    project_T(xk, wk_sb, bk_sb, khT)
    project_T(xq, wq_sb, bq_sb, qhT)

    # --- V: project to vhT then PE-transpose into vh[s, d] tiles ---
    vhT = sb.tile([128, seq], BF16, tag="vhT", bufs=1, name="vhT")
    project_T(xv, wv_sb, None, vhT)
    for j in range(nkt):
        tp = pps.tile([128, 128], BF16, tag="tp", bufs=2, name="tp")
        nc.tensor.transpose(tp, vhT[:, j * 128 : (j + 1) * 128], ident)
        for h in range(HPC):
            nc.vector.tensor_copy(
                out=vh[h][:, j * 65 : j * 65 + 64],
                in_=tp[:, h * 64 : (h + 1) * 64],
            )
    pps.release()
    ps = tc.alloc_tile_pool(name="ps", bufs=2, space="PSUM")

    if dbg is not None:
        nc.sync.dma_start(dbg["qhT"], qhT)
        nc.sync.dma_start(dbg["khT"], khT)
        nc.sync.dma_start(dbg["vh0"], vh[0])
        nc.sync.dma_start(dbg["vh1"], vh[1])

    # --- attention + output projection, per query block ---
    # Normalize/out-projection is deferred one q-block so the PE queue never
    # waits on the DVE reciprocal (head-of-line stalls re-throttle HAM).
    def norm_and_proj(av_sb, qb):
        avn = []
        for h in range(HPC):
            r_sb = sb.tile([128, QB], F32, tag="r", bufs=2, name="r_sb")
            nc.vector.reciprocal(out=r_sb[64:65, :], in_=av_sb[h][64:65, :])
            bc = ps.tile([64, QB], F32, tag="bc", bufs=1, name="bc")
            nc.tensor.matmul(
                bc,
                lhsT=ones_sb[64:65, :],
                rhs=r_sb[64:65, :],
                start=True,
                stop=True,
            )
            bc_sb = sb.tile([64, QB], F32, tag="bcs", bufs=2, name="bc_sb")
            nc.vector.tensor_copy(out=bc_sb, in_=bc)
            a = sb.tile([64, QB], BF16, tag=f"avn{h}", bufs=2, name=f"avn{h}")
            nc.vector.tensor_tensor(a, av_sb[h][0:64, :], bc_sb, MULT)
            avn.append(a)
            if dbg is not None:
                nc.sync.dma_start(dbg[f"den{h}"][qb : qb + 1, :], av_sb[h][64:65, :])
                nc.sync.dma_start(dbg[f"r{h}"][qb : qb + 1, :], r_sb[64:65, :])
                nc.sync.dma_start(dbg[f"bc{h}"][qb * 64 : (qb + 1) * 64, :], bc_sb)
                nc.sync.dma_start(
                    dbg[f"av{h}"][qb * 65 : (qb + 1) * 65, :], av_sb[h][0:65, :]
                )
        for st in range(QB // 128):
            ssl = slice(st * 128, (st + 1) * 128)
            op = ps.tile([128, QB], F32, tag="op", bufs=1, name="op")
            nc.tensor.matmul(
                op, lhsT=avn[0][:, ssl], rhs=wo0_sb, start=True, stop=False
            )
            nc.tensor.matmul(
                op, lhsT=avn[1][:, ssl], rhs=wo1_sb, start=False, stop=True
            )
            ost = sb.tile([128, QB], F32, tag="ost", bufs=3, name="ost")
            nc.vector.tensor_copy(out=ost, in_=op)
            nc.sync.dma_start(
                out_ap[qb * QB + st * 128 : qb * QB + (st + 1) * 128, :], ost
            )

    pending = None
    for qb in range(nqb):
        qsl = slice(qb * QB, (qb + 1) * QB)
        av = [
            ps.tile([128, QB], F32, tag=f"av{h}", bufs=1, name=f"av{h}")
            for h in range(HPC)
        ]
        for ktp in range(nkt // 2):
            # both heads' score tiles; h0/h1 matmuls interleaved so adjacent
            # PE instructions hit disjoint row groups and run concurrently
            sc = [
                ps.tile([128, 2 * QB], F32, tag="sc", bufs=2, name=f"sc{h}")
                for h in range(HPC)
            ]
            for half in range(2):
                kt_i = 2 * ktp + half
                for h in range(HPC):
                    hp = slice(h * 64, (h + 1) * 64)
                    nc.tensor.matmul(
                        sc[h][:, half * QB : (half + 1) * QB],
                        lhsT=khT[hp, kt_i * KT : (kt_i + 1) * KT],
                        rhs=qhT[hp, qsl],
                        start=True,
                        stop=True,
                    )
            ex = []
            for h in range(HPC):
                e = sb.tile([128, 2 * QB], BF16, tag="ex", bufs=4, name="ex")
                nc.scalar.activation(e, sc[h], EXP)
                ex.append(e)
            for h in range(HPC):
                for half in range(2):
                    kt_i = 2 * ktp + half
                    nc.tensor.matmul(
                        av[h][0:65, :],
                        lhsT=vh[h][:, kt_i * 65 : kt_i * 65 + 65],
                        rhs=ex[h][:, half * QB : (half + 1) * QB],
                        start=(kt_i == 0),
                        stop=(kt_i == nkt - 1),
                    )
        av_sb = []
        for h in range(HPC):
            asb = sb.tile([128, QB], F32, tag=f"avsb{h}", bufs=2, name=f"avsb{h}")
            nc.vector.tensor_copy(out=asb[0:65, :], in_=av[h][0:65, :])
            av_sb.append(asb)
        if pending is not None:
            norm_and_proj(*pending)
        pending = (av_sb, qb)
    norm_and_proj(*pending)

    ps.release()
    sb.release()
    const.release()


def build_bass(seq=S, debug_outs=False):
    nc = bacc.Bacc(
        "TRN2",
        debug=False,
        enable_asserts=False,
        target_bir_lowering=False,
    )
    ins = {}
    shapes = {
        "qt": (D, seq), "kt": (D, seq), "vt": (D, seq),
        "wq": (D, HD), "wk": (D, HD), "wv": (D, HD),
        "wo0": (64, D), "wo1": (64, D),
        "bq": (HD, 1), "bk": (HD, 1),
    }
    bf16_names = {"qt", "kt", "vt", "wq", "wk", "wv", "wo0", "wo1"}
    for name, shape in shapes.items():
        dt = BF16 if name in bf16_names else F32
        ins[name] = nc.dram_tensor(name, list(shape), dt, kind="ExternalInput").ap()
    out = nc.dram_tensor("out", [seq, D], F32, kind="ExternalOutput").ap()
    dbg = None
    if debug_outs:
        nkt, nqb = seq // KT, seq // QB
        dbg_shapes = {
            "qhT": ((128, seq), BF16), "khT": ((128, seq), BF16),
            "vh0": ((128, nkt * 65), BF16), "vh1": ((128, nkt * 65), BF16),
            "den0": ((nqb, QB), F32), "den1": ((nqb, QB), F32),
            "r0": ((nqb, QB), F32), "r1": ((nqb, QB), F32),
            "bc0": ((nqb * 64, QB), F32), "bc1": ((nqb * 64, QB), F32),
            "av0": ((nqb * 65, QB), F32), "av1": ((nqb * 65, QB), F32),
        }
        dbg = {
            n: nc.dram_tensor(f"dbg_{n}", list(sh), dt, kind="ExternalOutput").ap()
            for n, (sh, dt) in dbg_shapes.items()
        }
    with tile.TileContext(nc) as tc:
        mha_tile_kernel(tc, out, ins, seq=seq, dbg=dbg)
    nc.compile()
    return nc


def shard_inputs(q, k, v, Wq, bq, Wk, bk, Wv, bv, Wo, bo, seq=S):
    """Host-side shard prep. Returns (in_maps, const_vec)."""
    scale = 1.0 / np.sqrt(np.float32(DK))
    q, k, v = (np.asarray(x, np.float32) for x in (q, k, v))
    Wq, bq, Wk, bk, Wv, bv, Wo, bo = (
        np.asarray(x, np.float32) for x in (Wq, bq, Wk, bk, Wv, bv, Wo, bo)
    )
    bf = lambda x: np.ascontiguousarray(x).astype(NPBF16)
    in_maps = []
    for c in range(N_CORES):
        b = c // 4
        rows = slice(128 * (c % 4), 128 * (c % 4) + 128)
        in_maps.append({
            "qt": bf(q[b].T),
            "kt": bf(k[b].T),
            "vt": bf(v[b].T),
            "wq": bf((Wq[rows, :] * scale).T),
            "wk": bf(Wk[rows, :].T),
            "wv": bf(Wv[rows, :].T),
            "wo0": bf(Wo[:, rows][:, 0:64].T),
            "wo1": bf(Wo[:, rows][:, 64:128].T),
            "bq": np.ascontiguousarray((bq[rows] * scale).reshape(HD, 1)),
            "bk": np.ascontiguousarray(bk[rows].reshape(HD, 1)),
        })
    const_vec = (bv @ Wo.T + bo).astype(np.float32)
    return in_maps, const_vec


_NC_CACHE = {}


def run(inputs, seq=S, trace=False, trace_kwargs=None):
    if seq not in _NC_CACHE:
        _NC_CACHE[seq] = build_bass(seq=seq)
    nc = _NC_CACHE[seq]
    in_maps, const_vec = shard_inputs(**inputs, seq=seq)
    res = run_bass_kernel_spmd(
        nc,
        in_maps,
        core_ids=list(range(N_CORES)),
        trace=trace,
        **(trace_kwargs or {}),
    )
    out = np.zeros((B, seq, D), dtype=np.float32)
    for c in range(N_CORES):
        out[c // 4] += res.results[c]["out"]
    out += const_vec[None, None, :]
    return out, res


def kernel(**inputs):
    out, _ = run(inputs)
    return out


# revision 22
# speedup vs baseline: 1.6288x; 1.0760x over previous
"""Multi-head attention (B=2, S=4096, D=512, H=8) on 8 Trainium2 NeuronCores.

Sharding: batch x head-pair parallelism. Core c handles batch b = c // 4 and
heads {2*(c%4), 2*(c%4)+1} (128 contiguous rows of the QKV projection
weights, Megatron column-parallel; Wo row-parallel with the partial-sum
reduction done on the host at gather time).

Per-core device program (identical on all cores, different data; matmul
operands in bf16, all accumulation in fp32 PSUM):
  1. Project K, Q: khT/qhT [128(hd), 4096] = W @ x.T via 4 contraction chunks.
     The 1/sqrt(DK) score scale and biases are folded into Wq/bq host-side.
  2. Project V the same way, then PE-transpose to vh [4096(s), 64] per head,
     augmented with a ones column (65th) so the AV matmul also produces the
     softmax denominator.
  3. Attention, per 512-wide query block, per head, per pair of 128-wide key
     tiles (paired so each ACT exp call covers 1024 elements of free dim):
     scoresT [128(k), 512(q)] = kh_tile @ qh_block        (PE)
     expT = exp(scoresT) over both tiles of the pair      (ACT, PSUM->SBUF)
     av [65, 512] += vh_aug_tile.T @ expT                 (PE accumulate;
                                         row 64 = sum_k exp = denominator)
  4. Normalize: recip(denominator) (DVE), broadcast across 64 partitions
     (GpSimd), multiply (DVE) -> avn [64, 512] bf16 per head.
  5. Output projection: out[s,:] += avn_h.T @ WoT_h for both heads (K=64
     accumulating matmuls), DVE-evacuate, DMA to HBM.

Host gathers: out[b] = sum of the 4 per-core partials + bv @ Wo.T + bo.
"""

import ml_dtypes
import numpy as np

import concourse.mybir as mybir
import concourse.tile as tile
from concourse import bacc
from concourse.bass_utils import run_bass_kernel_spmd
from concourse.masks import make_identity

F32 = mybir.dt.float32
BF16 = mybir.dt.bfloat16
EXP = mybir.ActivationFunctionType.Exp
ADD = mybir.AluOpType.add
MULT = mybir.AluOpType.mult
NPBF16 = ml_dtypes.bfloat16

B, S, D, H = 2, 4096, 512, 8
DK = D // H          # 64
HPC = 2              # heads per core
HD = HPC * DK        # 128 head-dims per core
N_CORES = 8
QB = 512             # query block (matmul free dim)
KT = 128             # key tile (partition dim)
NCH = D // 128       # 4 contraction chunks for the projections


def mha_tile_kernel(tc, out_ap, ins, seq=S, dbg=None):
    """Emit the per-core MHA program. `ins` maps name -> DRAM AP."""
    nc = tc.nc
    nqb, nkt = seq // QB, seq // KT

    xq, xk, xv = ins["qt"], ins["kt"], ins["vt"]
    const = tc.alloc_tile_pool(name="const", bufs=1)
    sb = tc.alloc_tile_pool(name="sb", bufs=2)
    pps = tc.alloc_tile_pool(name="pps", bufs=2, space="PSUM")

    # --- constants ---
    wq_sb = const.tile([128, NCH, 128], BF16, tag="wq", name="wq_sb")
    wk_sb = const.tile([128, NCH, 128], BF16, tag="wk", name="wk_sb")
    wv_sb = const.tile([128, NCH, 128], BF16, tag="wv", name="wv_sb")
    for w_sb, name in ((wq_sb, "wq"), (wk_sb, "wk"), (wv_sb, "wv")):
        nc.sync.dma_start(w_sb, ins[name].rearrange("(c p) m -> p c m", p=128))
    wo0_sb = const.tile([64, QB], BF16, tag="wo0", name="wo0_sb")
    wo1_sb = const.tile([64, QB], BF16, tag="wo1", name="wo1_sb")
    nc.sync.dma_start(wo0_sb, ins["wo0"])
    nc.sync.dma_start(wo1_sb, ins["wo1"])
    bq_sb = const.tile([128, 1], F32, tag="bq", name="bq_sb")
    bk_sb = const.tile([128, 1], F32, tag="bk", name="bk_sb")
    nc.sync.dma_start(bq_sb, ins["bq"])
    nc.sync.dma_start(bk_sb, ins["bk"])

    ident = const.tile([128, 128], BF16, tag="ident", name="ident")
    make_identity(nc, ident)
    ones_sb = const.tile([128, 64], F32, tag="ones", name="ones_sb")
    nc.vector.memset(ones_sb, 1.0)

    # --- persistent activations ---
    qhT = const.tile([128, seq], BF16, tag="qhT", name="qhT")
    khT = const.tile([128, seq], BF16, tag="khT", name="khT")
    # vh per head: nkt tiles of [128, 65]; column 64 is the ones column.
    vh = [
        const.tile([128, nkt * 65], BF16, tag=f"vh{h}", name=f"vh{h}")
        for h in range(HPC)
    ]
    for h in range(HPC):
        ones_col = vh[h].rearrange("p (j c) -> p j c", c=65)[:, :, 64]
        nc.vector.tensor_copy(out=ones_col, in_=ones_sb[:, 0:nkt])

    # --- projections of K and Q: dstT[hd, s] = W @ x.T (+ bias) ---
    def project_T(x_dram, w_sb, bias, dstT):
        xc = [
            sb.tile([128, seq], BF16, tag="xchunk", bufs=NCH + 1, name=f"xc{c}")
            for c in range(NCH)
        ]
        for c in range(NCH):
            nc.sync.dma_start(xc[c], x_dram[c * 128 : (c + 1) * 128, :])
        for qb in range(nqb):
            acc = pps.tile([128, QB], F32, tag="proj", bufs=4, name="prj")
            for c in range(NCH):
                nc.tensor.matmul(
                    acc,
                    lhsT=w_sb[:, c, :],
                    rhs=xc[c][:, qb * QB : (qb + 1) * QB],
                    start=(c == 0),
                    stop=(c == NCH - 1),
                )
            dst = dstT[:, qb * QB : (qb + 1) * QB]
            if bias is None:
                nc.vector.tensor_copy(out=dst, in_=acc)
            else:
                nc.vector.tensor_scalar(dst, acc, bias[:, 0:1], None, ADD)

    project_T(xk, wk_sb, bk_sb, khT)
    project_T(xq, wq_sb, bq_sb, qhT)

    # --- V: project to vhT then PE-transpose into vh[s, d] tiles ---
    vhT = sb.tile([128, seq], BF16, tag="vhT", bufs=1, name="vhT")
    project_T(xv, wv_sb, None, vhT)
    for j in range(nkt):
        tp = pps.tile([128, 128], BF16, tag="tp", bufs=2, name="tp")
        nc.tensor.transpose(tp, vhT[:, j * 128 : (j + 1) * 128], ident)
        for h in range(HPC):
            nc.vector.tensor_copy(
                out=vh[h][:, j * 65 : j * 65 + 64],
                in_=tp[:, h * 64 : (h + 1) * 64],
            )
    pps.release()
    ps = tc.alloc_tile_pool(name="ps", bufs=2, space="PSUM")

    if dbg is not None:
        nc.sync.dma_start(dbg["qhT"], qhT)
        nc.sync.dma_start(dbg["khT"], khT)
        nc.sync.dma_start(dbg["vh0"], vh[0])
        nc.sync.dma_start(dbg["vh1"], vh[1])

    # --- attention + output projection, per query block ---
    # Normalize/out-projection is deferred one q-block so the PE queue never
    # waits on the DVE reciprocal (head-of-line stalls re-throttle HAM).
    def norm_and_proj(av_sb, qb):
        avn = []
        for h in range(HPC):
            r_sb = sb.tile([128, QB], F32, tag="r", bufs=2, name="r_sb")
            nc.vector.reciprocal(out=r_sb[64:65, :], in_=av_sb[h][64:65, :])
            bc = ps.tile([64, QB], F32, tag="bc", bufs=1, name="bc")
            nc.tensor.matmul(
                bc,
                lhsT=ones_sb[64:65, :],
                rhs=r_sb[64:65, :],
                start=True,
                stop=True,
            )
            bc_sb = sb.tile([64, QB], F32, tag="bcs", bufs=2, name="bc_sb")
            nc.vector.tensor_copy(out=bc_sb, in_=bc)
            a = sb.tile([64, QB], BF16, tag=f"avn{h}", bufs=2, name=f"avn{h}")
            nc.vector.tensor_tensor(a, av_sb[h][0:64, :], bc_sb, MULT)
            avn.append(a)
            if dbg is not None:
                nc.sync.dma_start(dbg[f"den{h}"][qb : qb + 1, :], av_sb[h][64:65, :])
                nc.sync.dma_start(dbg[f"r{h}"][qb : qb + 1, :], r_sb[64:65, :])
                nc.sync.dma_start(dbg[f"bc{h}"][qb * 64 : (qb + 1) * 64, :], bc_sb)
                nc.sync.dma_start(
                    dbg[f"av{h}"][qb * 65 : (qb + 1) * 65, :], av_sb[h][0:65, :]
                )
        for st in range(QB // 128):
            ssl = slice(st * 128, (st + 1) * 128)
            op = ps.tile([128, QB], F32, tag="op", bufs=1, name="op")
            nc.tensor.matmul(
                op, lhsT=avn[0][:, ssl], rhs=wo0_sb, start=True, stop=False
            )
            nc.tensor.matmul(
                op, lhsT=avn[1][:, ssl], rhs=wo1_sb, start=False, stop=True
            )
            ost = sb.tile([128, QB], F32, tag="ost", bufs=3, name="ost")
            nc.vector.tensor_copy(out=ost, in_=op)
            nc.sync.dma_start(
                out_ap[qb * QB + st * 128 : qb * QB + (st + 1) * 128, :], ost
            )

    KPG = 2  # key tiles per exp group (one ACT call covers KPG*QB elements)
    pending = None
    for qb in range(nqb):
        qsl = slice(qb * QB, (qb + 1) * QB)
        av = [
            ps.tile([128, QB], F32, tag=f"av{h}", bufs=1, name=f"av{h}")
            for h in range(HPC)
        ]

        def av_group(ktg, ex):
            for h in range(HPC):
                for part in range(KPG):
                    kt_i = KPG * ktg + part
                    nc.tensor.matmul(
                        av[h][0:65, :],
                        lhsT=vh[h][:, kt_i * 65 : kt_i * 65 + 65],
                        rhs=ex[h][:, part * QB : (part + 1) * QB],
                        start=(kt_i == 0),
                        stop=(kt_i == nkt - 1),
                    )

        # AV matmuls run one key-group behind the score matmuls so the PE
        # stream never waits on ACT exp (PE idle gaps re-throttle HAM).
        prev_av = None
        for ktg in range(nkt // KPG):
            # both heads' score super-tiles (KPG key tiles each); h0/h1
            # matmuls interleaved so adjacent PE instructions hit disjoint
            # row groups and run concurrently
            sc = [
                ps.tile([128, KPG * QB], F32, tag="sc", bufs=2, name=f"sc{h}")
                for h in range(HPC)
            ]
            for part in range(KPG):
                kt_i = KPG * ktg + part
                for h in range(HPC):
                    hp = slice(h * 64, (h + 1) * 64)
                    nc.tensor.matmul(
                        sc[h][:, part * QB : (part + 1) * QB],
                        lhsT=khT[hp, kt_i * KT : (kt_i + 1) * KT],
                        rhs=qhT[hp, qsl],
                        start=True,
                        stop=True,
                    )
            ex = []
            for h in range(HPC):
                e = sb.tile([128, KPG * QB], BF16, tag="ex", bufs=6, name="ex")
                nc.scalar.activation(e, sc[h], EXP)
                ex.append(e)
            if prev_av is not None:
                av_group(*prev_av)
            prev_av = (ktg, ex)
        av_group(*prev_av)
        av_sb = []
        for h in range(HPC):
            asb = sb.tile([128, QB], F32, tag=f"avsb{h}", bufs=2, name=f"avsb{h}")
            nc.vector.tensor_copy(out=asb[0:65, :], in_=av[h][0:65, :])
            av_sb.append(asb)
        if pending is not None:
            norm_and_proj(*pending)
        pending = (av_sb, qb)
    norm_and_proj(*pending)

    ps.release()
    sb.release()
    const.release()


def build_bass(seq=S, debug_outs=False):
    nc = bacc.Bacc(
        "TRN2",
        debug=False,
        enable_asserts=False,
        target_bir_lowering=False,
    )
    ins = {}
    shapes = {
        "qt": (D, seq), "kt": (D, seq), "vt": (D, seq),
        "wq": (D, HD), "wk": (D, HD), "wv": (D, HD),
        "wo0": (64, D), "wo1": (64, D),
        "bq": (HD, 1), "bk": (HD, 1),
    }
    bf16_names = {"qt", "kt", "vt", "wq", "wk", "wv", "wo0", "wo1"}
    for name, shape in shapes.items():
        dt = BF16 if name in bf16_names else F32
        ins[name] = nc.dram_tensor(name, list(shape), dt, kind="ExternalInput").ap()
    out = nc.dram_tensor("out", [seq, D], F32, kind="ExternalOutput").ap()
    dbg = None
    if debug_outs:
        nkt, nqb = seq // KT, seq // QB
        dbg_shapes = {
            "qhT": ((128, seq), BF16), "khT": ((128, seq), BF16),
            "vh0": ((128, nkt * 65), BF16), "vh1": ((128, nkt * 65), BF16),
            "den0": ((nqb, QB), F32), "den1": ((nqb, QB), F32),
            "r0": ((nqb, QB), F32), "r1": ((nqb, QB), F32),
            "bc0": ((nqb * 64, QB), F32), "bc1": ((nqb * 64, QB), F32),
            "av0": ((nqb * 65, QB), F32), "av1": ((nqb * 65, QB), F32),
        }
        dbg = {
            n: nc.dram_tensor(f"dbg_{n}", list(sh), dt, kind="ExternalOutput").ap()
            for n, (sh, dt) in dbg_shapes.items()
        }
    with tile.TileContext(nc) as tc:
        mha_tile_kernel(tc, out, ins, seq=seq, dbg=dbg)
    nc.compile()
    return nc


def shard_inputs(q, k, v, Wq, bq, Wk, bk, Wv, bv, Wo, bo, seq=S):
    """Host-side shard prep. Returns (in_maps, const_vec)."""
    scale = 1.0 / np.sqrt(np.float32(DK))
    q, k, v = (np.asarray(x, np.float32) for x in (q, k, v))
    Wq, bq, Wk, bk, Wv, bv, Wo, bo = (
        np.asarray(x, np.float32) for x in (Wq, bq, Wk, bk, Wv, bv, Wo, bo)
    )
    bf = lambda x: np.ascontiguousarray(x).astype(NPBF16)
    in_maps = []
    for c in range(N_CORES):
        b = c // 4
        rows = slice(128 * (c % 4), 128 * (c % 4) + 128)
        in_maps.append({
            "qt": bf(q[b].T),
            "kt": bf(k[b].T),
            "vt": bf(v[b].T),
            "wq": bf((Wq[rows, :] * scale).T),
            "wk": bf(Wk[rows, :].T),
            "wv": bf(Wv[rows, :].T),
            "wo0": bf(Wo[:, rows][:, 0:64].T),
            "wo1": bf(Wo[:, rows][:, 64:128].T),
            "bq": np.ascontiguousarray((bq[rows] * scale).reshape(HD, 1)),
            "bk": np.ascontiguousarray(bk[rows].reshape(HD, 1)),
        })
    const_vec = (bv @ Wo.T + bo).astype(np.float32)
    return in_maps, const_vec


_NC_CACHE = {}


def run(inputs, seq=S, trace=False, trace_kwargs=None):
    if seq not in _NC_CACHE:
        _NC_CACHE[seq] = build_bass(seq=seq)
    nc = _NC_CACHE[seq]
    in_maps, const_vec = shard_inputs(**inputs, seq=seq)
    res = run_bass_kernel_spmd(
        nc,
        in_maps,
        core_ids=list(range(N_CORES)),
        trace=trace,
        **(trace_kwargs or {}),
    )
    out = np.zeros((B, seq, D), dtype=np.float32)
    for c in range(N_CORES):
        out[c // 4] += res.results[c]["out"]
    out += const_vec[None, None, :]
    return out, res


def kernel(**inputs):
    out, _ = run(inputs)
    return out
